# revision 2
# baseline (speedup 1.0000x reference)
"""DIFFormerConv (simple linear attention + dense GCN) on 8 trn2 NeuronCores.

v5 on top of v4 (scheduling/overlap pass):
  - A1 software pipelining: batch b's kvs/sums/AR-staging is emitted after
    batch b+1's projections, so the PE never head-of-line blocks on the
    vector-engine norm chain
  - vmean staged in ONE SBUF tile, flushed with ONE DMA; AllGather fired
    before batch 7's kvs group (its input is ready earlier than the
    AllReduce's)
  - GCN emitted before the attention epilogue (its AllGather dependency
    resolves first); gathered vm loaded in 8 big DMAs (one per source
    core), arn streamed from the scalar queue -- kills ~80 serialized
    600ns DMA issues on the sync engine
  - epilogue: one batched kpk load, vs/ksp staging on the gpsimd queue,
    denominator chain split per 4-batch tile for earlier pat start
  - AR staging writes issued from the gpsimd queue

Math identical to v4 (bf16 + fp8 DoubleRow GCN with x1024 arn scale).
"""

import sys

sys.path.insert(0, "/opt/trn_rl_repo")

import numpy as np
import ml_dtypes

from concourse import bass, bacc, tile, mybir
from concourse.bass_utils import run_bass_kernel_spmd

B, C, N, H, D = 8, 256, 4096, 4, 64
NCORES = 8
S = N // NCORES          # 512 nodes per core
HD = H * D               # 256
F32 = mybir.dt.float32
BF16 = mybir.dt.bfloat16
FP8 = mybir.dt.float8e4
AX = mybir.AxisListType.X
ALU = mybir.AluOpType
ACTF = mybir.ActivationFunctionType
PERF_DR = mybir.MatmulPerfMode.DoubleRow
RG = [list(range(NCORES))]

ASCALE = 1024.0

_CACHE = {}


def _indicators():
    i4 = np.zeros((128, 8), np.float32)
    for p in range(128):
        i4[p, 0 + p // 64] = 1.0
        i4[p, 4 + 2 + p // 64] = 1.0
    sel = np.zeros((128, 16 * 128), np.float32)
    for b in range(B):
        for hc in range(2):
            k = 2 * b + hc
            r0 = 32 * (b % 4) + 2 * hc
            sel[r0, 128 * k: 128 * k + 64] = 1.0
            sel[r0 + 1, 128 * k + 64: 128 * k + 128] = 1.0
    return i4, sel


def _build():
    nc = bacc.Bacc("TRN2", target_bir_lowering=False, debug=False,
                   num_devices=NCORES)

    xq = nc.dram_tensor("xq", [B, 2, 128, S], BF16, kind="ExternalInput")
    xs = nc.dram_tensor("xs", [B, 2, 128, S], BF16, kind="ExternalInput")
    arn_d = nc.dram_tensor("arn", [16, 128, 2 * S], FP8, kind="ExternalInput")
    wkv = nc.dram_tensor("wkv", [2, 128, 2 * HD], BF16, kind="ExternalInput")
    wq = nc.dram_tensor("wq", [2, 128, HD], BF16, kind="ExternalInput")
    bkv = nc.dram_tensor("bkv", [1, 2 * HD], BF16, kind="ExternalInput")
    bq = nc.dram_tensor("bq", [2, 128, 1], F32, kind="ExternalInput")
    i4_d = nc.dram_tensor("i4_in", [128, 8], BF16, kind="ExternalInput")
    sel_d = nc.dram_tensor("sel_in", [128, 16 * 128], BF16,
                           kind="ExternalInput")
    ones_c_d = nc.dram_tensor("ones_c", [128, 1], BF16, kind="ExternalInput")
    ones_r_d = nc.dram_tensor("ones_r", [1, 128], BF16, kind="ExternalInput")
    out = nc.dram_tensor("out", [B, D, S], F32, kind="ExternalOutput")

    with nc.allow_low_precision(reason="bf16/fp8 compute intentional"), \
            tile.TileContext(nc) as tc:
        with (
            tc.tile_pool(name="pers", bufs=1) as pp,
            tc.tile_pool(name="work", bufs=2) as wk,
            tc.tile_pool(name="dram", bufs=1, space="DRAM") as dp,
        ):
            ar_in = dp.tile([B, 2, 130, 66], BF16, tag="ar_in", name="ar_in")
            ar_out = dp.tile([B, 2, 130, 66], BF16, tag="ar_out",
                             name="ar_out", addr_space="Shared")
            vm_loc = dp.tile([4, 128, 512], FP8, tag="vm_loc",
                             name="vm_loc")
            vm_all = dp.tile([NCORES, 4, 128, 512], FP8, tag="vm_all",
                             name="vm_all", addr_space="Shared")

            # ---- constants ----
            wkv_t = [pp.tile([128, 2 * HD], BF16, tag=f"wkv{c}", name=f"wkv{c}")
                     for c in range(2)]
            wq_t = [pp.tile([128, HD], BF16, tag=f"wq{c}", name=f"wq{c}")
                    for c in range(2)]
            for c in range(2):
                nc.sync.dma_start(out=wkv_t[c][:], in_=wkv[c])
                nc.sync.dma_start(out=wq_t[c][:], in_=wq[c])
            bkv_row = pp.tile([1, 2 * HD], BF16, tag="bkvrow")
            nc.sync.dma_start(out=bkv_row[:], in_=bkv[:])
            bq_col = [pp.tile([128, 1], F32, tag=f"bqc{c}", name=f"bqc{c}")
                      for c in range(2)]
            for c in range(2):
                nc.sync.dma_start(out=bq_col[c][:], in_=bq[c])
            i4_t = pp.tile([128, 8], BF16, tag="i4")
            nc.sync.dma_start(out=i4_t[:], in_=i4_d[:])
            sel_t = pp.tile([128, 16 * 128], BF16, tag="sel")
            nc.sync.dma_start(out=sel_t[:], in_=sel_d[:])
            ones_col = pp.tile([128, 1], BF16, tag="ones_col")
            nc.sync.dma_start(out=ones_col[:], in_=ones_c_d[:])
            ones_row = pp.tile([1, 128], BF16, tag="ones_row")
            nc.sync.dma_start(out=ones_row[:], in_=ones_r_d[:])

            # persistent SBUF tensors
            q_bf = [[pp.tile([128, S], BF16, tag=f"q{b}_{hc}", name=f"q{b}_{hc}")
                     for hc in range(2)] for b in range(B)]
            # vmean staging: one tile, col = (pg*2+sbp)*256 + 128*(sb%2)
            #                + 64*(b%2) + d
            vm_big = pp.tile([128, 2048], FP8, tag="vm_big")
            attn_sb = [pp.tile([128, S], BF16, tag=f"at{p}", name=f"at{p}")
                       for p in range(4)]

            # =============== pass A1 (software-pipelined) ===============
            with tc.tile_pool(name="psA", bufs=1, space="PSUM") as psA:
                kvbf_all = {}
                knt_all = {}

                def emit_proj(b):
                    xs0 = wk.tile([128, S], BF16, tag="xs0", bufs=2)
                    xs1 = wk.tile([128, S], BF16, tag="xs1", bufs=2)
                    nc.sync.dma_start(out=xs0[:], in_=xs[b, 0])
                    nc.sync.dma_start(out=xs1[:], in_=xs[b, 1])
                    kvbf = [wk.tile([128, 2 * HD + 1], BF16, tag=f"kvbf{sb}",
                                    bufs=2, name=f"kvbf{sb}")
                            for sb in range(4)]
                    knt = [wk.tile([128, HD], BF16, tag=f"knt{sb}",
                                   bufs=2, name=f"knt{sb}") for sb in range(4)]
                    ssk_all = wk.tile([128, 16], F32, tag="ssk_all", bufs=2)
                    for sb in range(4):
                        sl = slice(sb * 128, (sb + 1) * 128)
                        pkv = psA.tile([128, 2 * HD], F32, tag="pkv", bufs=3)
                        nc.tensor.matmul(pkv[:], lhsT=xs0[:, sl],
                                         rhs=wkv_t[0][:], start=True,
                                         stop=False)
                        nc.tensor.matmul(pkv[:], lhsT=xs1[:, sl],
                                         rhs=wkv_t[1][:], start=False,
                                         stop=False)
                        nc.tensor.matmul(pkv[:], lhsT=ones_row[:],
                                         rhs=bkv_row[:], start=False,
                                         stop=True)
                        nc.scalar.activation(kvbf[sb][:, 0:2 * HD], pkv[:],
                                             ACTF.Copy)
                        nc.gpsimd.tensor_copy(
                            out=kvbf[sb][:, 2 * HD:2 * HD + 1],
                            in_=ones_col[:])
                        sq = wk.tile([128, HD], BF16, tag="sq", bufs=2)
                        nc.scalar.activation(sq[:], kvbf[sb][:, 0:HD],
                                             ACTF.Square)
                        nc.vector.reduce_sum(
                            ssk_all[:, 4 * sb:4 * sb + 4],
                            sq[:].rearrange("p (h d) -> p h d", h=H),
                            axis=AX)
                        vcol = (((b // 2) * 2 + sb // 2) * 256
                                + 128 * (sb % 2) + (b % 2) * D)
                        nc.vector.reduce_sum(
                            vm_big[:, vcol:vcol + D],
                            kvbf[sb][:, HD:2 * HD].rearrange(
                                "p (h d) -> p d h", h=H),
                            axis=AX)
                    snk = wk.tile([128, 16], F32, tag="snk", bufs=2)
                    nc.scalar.activation(snk[:], ssk_all[:], ACTF.Sqrt)
                    rk = wk.tile([128, 16], BF16, tag="rk", bufs=2)
                    nc.vector.reciprocal(rk[:], snk[:])
                    for sb in range(4):
                        nc.vector.tensor_tensor(
                            out=knt[sb][:].rearrange("p (h d) -> p h d", h=H),
                            in0=kvbf[sb][:, 0:HD].rearrange(
                                "p (h d) -> p h d", h=H),
                            in1=rk[:, 4 * sb:4 * sb + 4].to_broadcast(
                                [128, H, D]),
                            op=ALU.mult)
                    kvbf_all[b] = kvbf
                    knt_all[b] = knt

                def emit_kvs(b):
                    kvbf = kvbf_all.pop(b)
                    knt = knt_all.pop(b)
                    kvs0 = psA.tile([128, HD + 1], F32, tag="kvs0", bufs=1)
                    kvs1 = psA.tile([128, HD + 1], F32, tag="kvs1", bufs=1)
                    vsr = psA.tile([1, HD], F32, tag="vsr", bufs=1)
                    for sb in range(4):
                        nc.tensor.matmul(kvs0[:], lhsT=knt[sb][:, 0:128],
                                         rhs=kvbf[sb][:, HD:2 * HD + 1],
                                         start=(sb == 0), stop=(sb == 3))
                    for sb in range(4):
                        nc.tensor.matmul(kvs1[:], lhsT=knt[sb][:, 128:HD],
                                         rhs=kvbf[sb][:, HD:2 * HD + 1],
                                         start=(sb == 0), stop=(sb == 3))
                    for sb in range(4):
                        nc.tensor.matmul(vsr[:], lhsT=ones_col[:],
                                         rhs=kvbf[sb][:, HD:2 * HD],
                                         start=(sb == 0), stop=(sb == 3))
                    for hc in range(2):
                        kvs_ps = kvs0 if hc == 0 else kvs1
                        off = hc * 128
                        ast = wk.tile([128, 66], BF16, tag=f"ast{hc}", bufs=2,
                                      name=f"ast{hc}")
                        nc.scalar.activation(ast[0:64, 0:64],
                                             kvs_ps[0:64, off:off + 64],
                                             ACTF.Copy)
                        nc.scalar.activation(ast[64:128, 0:64],
                                             kvs_ps[64:128, off + 64:off + 128],
                                             ACTF.Copy)
                        nc.scalar.activation(ast[:, 64:65],
                                             kvs_ps[:, HD:HD + 1],
                                             ACTF.Copy)
                        nc.sync.dma_start(out=ar_in[b, hc, 0:128, 0:66],
                                            in_=ast[:])
                    vsb = wk.tile([1, HD], BF16, tag="vsb", bufs=2)
                    nc.scalar.activation(vsb[:], vsr[:], ACTF.Copy)
                    nc.sync.dma_start(out=ar_in[b, 0, 128:130, 0:64],
                                      in_=vsb[0:1, 0:128])
                    nc.sync.dma_start(out=ar_in[b, 1, 128:130, 0:64],
                                      in_=vsb[0:1, 128:256])

                for b in range(B):
                    emit_proj(b)
                    if b >= 1:
                        emit_kvs(b - 1)
                    if b == B - 1:
                        # vmean complete: flush + fire AllGather now (its
                        # consumer, the GCN, unblocks before the epilogue)
                        for pg in range(4):
                            nc.sync.dma_start(
                                out=vm_loc[pg],
                                in_=vm_big[:, 512 * pg:512 * (pg + 1)])
                        nc.gpsimd.collective_compute(
                            "AllGather", ALU.bypass, ins=[vm_loc.opt()],
                            outs=[vm_all.opt()], replica_groups=RG)
                emit_kvs(B - 1)

            nc.gpsimd.collective_compute(
                "AllReduce", ALU.add, ins=[ar_in.opt()],
                outs=[ar_out.opt()], replica_groups=RG)

            # =============== pass A2: q projection + norms ===============
            sqn2 = [pp.tile([128, S], F32, tag=f"sqn{t}", name=f"sqn{t}")
                    for t in range(2)]
            for t in range(2):
                nc.vector.memset(sqn2[t][:], 0.0)
            with tc.tile_pool(name="psA2", bufs=1, space="PSUM") as psA2:
                for b in range(B):
                    xq0 = wk.tile([128, S], BF16, tag="xq0", bufs=2)
                    xq1 = wk.tile([128, S], BF16, tag="xq1", bufs=2)
                    nc.sync.dma_start(out=xq0[:], in_=xq[b, 0])
                    nc.sync.dma_start(out=xq1[:], in_=xq[b, 1])
                    ss = psA2.tile([4, S], F32, tag="ss", bufs=2)
                    for hc in range(2):
                        hsl = slice(hc * 128, (hc + 1) * 128)
                        psq = psA2.tile([128, S], F32, tag="psq", bufs=2)
                        nc.tensor.matmul(psq[:], lhsT=wq_t[0][:, hsl],
                                         rhs=xq0[:], start=True, stop=False)
                        nc.tensor.matmul(psq[:], lhsT=wq_t[1][:, hsl],
                                         rhs=xq1[:], start=False, stop=True)
                        nc.scalar.activation(q_bf[b][hc][:], psq[:],
                                             ACTF.Identity,
                                             bias=bq_col[hc][:])
                        qsq = wk.tile([128, S], BF16, tag="qsq", bufs=2)
                        nc.vector.tensor_mul(qsq[:], q_bf[b][hc][:],
                                             q_bf[b][hc][:])
                        nc.tensor.matmul(ss[:],
                                         lhsT=i4_t[:, 4 * hc:4 * hc + 4],
                                         rhs=qsq[:], start=(hc == 0),
                                         stop=(hc == 1))
                    r0 = 32 * (b % 4)
                    nc.scalar.activation(sqn2[b // 4][r0:r0 + 4, :], ss[:],
                                         ACTF.Sqrt)

            # =============== GCN: fp8 DoubleRow from gathered vm ==========
            # (emitted before the epilogue: AllGather resolves first)
            gcn_sb = [pp.tile([128, S], BF16, tag=f"gcn{pg}", name=f"gcn{pg}")
                      for pg in range(4)]
            vmb = [pp.tile([128, 2048], FP8, tag=f"vmb{c}", name=f"vmb{c}")
                   for c in range(NCORES)]
            for c in range(NCORES):
                nc.sync.dma_start(
                    out=vmb[c][:].rearrange("p (g cc) -> p g cc", g=4),
                    in_=vm_all[c].rearrange("g p cc -> p g cc"))
            with tc.tile_pool(name="psG", bufs=1, space="PSUM") as psG:
                pgn = [psG.tile([128, S], F32, tag=f"pg{pg}", name=f"pg{pg}")
                       for pg in range(4)]
                for gp in range(16):
                    arn_t = wk.tile([128, 2 * S], FP8, tag="arn", bufs=4)
                    nc.sync.dma_start(out=arn_t[:], in_=arn_d[gp])
                    for pg in range(4):
                        vcol = ((pg * 2) + (gp % 2)) * 256
                        nc.tensor.matmul(
                            pgn[pg][:],
                            lhsT=vmb[gp // 2][:, vcol:vcol + 256].rearrange(
                                "p (two m) -> p two m", two=2),
                            rhs=arn_t[:].rearrange("p (two n) -> p two n",
                                                   two=2),
                            start=(gp == 0), stop=(gp == 15),
                            perf_mode=PERF_DR)
                for pg in range(4):
                    nc.scalar.activation(gcn_sb[pg][:], pgn[pg][:], ACTF.Copy)

            # =============== phase 2: attention epilogue ===============
            with tc.tile_pool(name="psP", bufs=1, space="PSUM") as psP:
                # one batched load of all kvs/ks blocks
                kpk_big = pp.tile([128, 16 * 66], BF16, tag="kpk_big")
                for b in range(B):
                    nc.sync.dma_start(
                        out=kpk_big[:, 132 * b:132 * (b + 1)]
                        .rearrange("p (hc c) -> p hc c", hc=2),
                        in_=ar_out[b, :, 0:128, :].rearrange(
                            "hc p c -> p hc c"))
                vsp_all = pp.tile([128, 8 * 64], BF16, tag="vsp_all")
                nc.vector.memset(vsp_all[:], 0.0)
                for b in range(B):
                    for hc in range(2):
                        r0 = 32 * (b % 4) + 2 * hc
                        nc.sync.dma_start(
                            out=vsp_all[r0:r0 + 2, 64 * b:64 * b + 64],
                            in_=ar_out[b, hc, 128:130, 0:64])
                ksp_all = pp.tile([128, 64], BF16, tag="ksp_all")
                nc.vector.memset(ksp_all[:], 0.0)
                for b in range(B):
                    for hc in range(2):
                        c0 = 8 * b + 6 * hc
                        k16 = 2 * b + hc
                        nc.gpsimd.tensor_copy(
                            out=ksp_all[0:64, c0:c0 + 1],
                            in_=kpk_big[0:64, 66 * k16 + 64:66 * k16 + 65])
                        nc.gpsimd.tensor_copy(
                            out=ksp_all[64:128, c0 + 1:c0 + 2],
                            in_=kpk_big[64:128, 66 * k16 + 64:66 * k16 + 65])

                pden2 = [pp.tile([128, S], F32, tag=f"pden{t}", name=f"pden{t}")
                         for t in range(2)]
                for t in range(2):
                    nc.vector.memset(pden2[t][:], 1.0)
                rp_bf = [pp.tile([128, S], BF16, tag=f"rp_bf{t}",
                                 name=f"rp_bf{t}") for t in range(2)]
                vvr = [pp.tile([128, S], BF16, tag=f"vvr{t}",
                               name=f"vvr{t}") for t in range(2)]

                def emit_pden(b):
                    pden = psP.tile([4, S], F32, tag="pden", bufs=2)
                    for hc in range(2):
                        nc.tensor.matmul(
                            pden[:],
                            lhsT=ksp_all[:, 8 * b + 4 * hc:8 * b + 4 * hc + 4],
                            rhs=q_bf[b][hc][:],
                            start=(hc == 0), stop=(hc == 1))
                    r0 = 32 * (b % 4)
                    nc.scalar.activation(pden2[b // 4][r0:r0 + 4, :], pden[:],
                                         ACTF.Copy)

                def emit_chain(t):
                    dd = wk.tile([128, S], F32, tag="dd", bufs=2)
                    nc.vector.scalar_tensor_tensor(dd[:], in0=sqn2[t][:],
                                                   scalar=float(N),
                                                   in1=pden2[t][:],
                                                   op0=ALU.mult, op1=ALU.add)
                    rp = wk.tile([128, S], F32, tag="rp", bufs=2)
                    nc.vector.reciprocal(rp[:], dd[:])
                    nc.scalar.activation(rp_bf[t][:], rp[:], ACTF.Copy)
                    nc.vector.tensor_mul(vvr[t][:], sqn2[t][:], rp[:])

                def emit_pat(b):
                    pat = psP.tile([64, S], F32, tag="pat", bufs=2)
                    for hc in range(2):
                        k16 = 2 * b + hc
                        pbc = psP.tile([128, S], F32, tag="pbc", bufs=3)
                        nc.tensor.matmul(
                            pbc[:],
                            lhsT=sel_t[:, 128 * k16:128 * (k16 + 1)],
                            rhs=rp_bf[b // 4][:], start=True, stop=True)
                        qs = wk.tile([128, S], BF16, tag="qs", bufs=2)
                        qsc = wk.tile([128, S], BF16, tag="qsc", bufs=2)
                        nc.scalar.activation(qsc[:], pbc[:], ACTF.Copy)
                        nc.vector.tensor_mul(qs[:], q_bf[b][hc][:], qsc[:])
                        nc.tensor.matmul(
                            pat[:],
                            lhsT=kpk_big[:, 66 * k16:66 * k16 + 64],
                            rhs=qs[:], start=(hc == 0), stop=False)
                    nc.tensor.matmul(pat[:],
                                     lhsT=vsp_all[:, 64 * b:64 * b + 64],
                                     rhs=vvr[b // 4][:], start=False,
                                     stop=True)
                    nc.scalar.activation(
                        attn_sb[b // 2][(b % 2) * D:(b % 2 + 1) * D, :],
                        pat[:], ACTF.Copy, scale=0.25)

                for b in range(4):
                    emit_pden(b)
                emit_chain(0)
                for b in range(4, 8):
                    emit_pden(b)
                for b in range(4):
                    emit_pat(b)
                emit_chain(1)
                for b in range(4, 8):
                    emit_pat(b)

            # =============== final: add GCN + attention ===============
            for p in range(4):
                ot = wk.tile([128, S], F32, tag="ot", bufs=2)
                nc.vector.scalar_tensor_tensor(ot[:], in0=gcn_sb[p][:],
                                               scalar=1.0 / ASCALE,
                                               in1=attn_sb[p][:],
                                               op0=ALU.mult, op1=ALU.add)
                nc.sync.dma_start(out=out[2 * p], in_=ot[0:D, :])
                nc.sync.dma_start(out=out[2 * p + 1], in_=ot[D:128, :])
    nc.compile()
    return nc


def _prep_inputs(query_input, source_input, adj, Wq_w, Wq_b, Wk_w, Wk_b,
                 Wv_w, Wv_b):
    bf = ml_dtypes.bfloat16
    f8 = ml_dtypes.float8_e4m3fn
    xq_np = np.asarray(query_input, dtype=np.float32)
    xs_np = np.asarray(source_input, dtype=np.float32)
    adj_np = np.asarray(adj, dtype=np.float32)

    arnT = np.ascontiguousarray(adj_np.T)
    np.fill_diagonal(arnT, arnT.diagonal() + 1.0)
    colscale = (ASCALE * 0.25 / (adj_np.sum(axis=1) + 1.0)).astype(np.float32)
    arnT *= colscale[None, :]
    arn_c = arnT.astype(f8)

    wk_T = np.asarray(Wk_w, np.float32).T
    wv_T = np.asarray(Wv_w, np.float32).T
    wkv_h = np.concatenate([wk_T, wv_T], axis=1).reshape(2, 128, 2 * HD)
    wq_h = np.ascontiguousarray(
        np.asarray(Wq_w, np.float32).T).reshape(2, 128, HD)
    bkv_h = np.concatenate([np.asarray(Wk_b, np.float32),
                            np.asarray(Wv_b, np.float32)]).reshape(1, 2 * HD)
    bq_h = np.asarray(Wq_b, np.float32).reshape(2, 128, 1)

    i4, sel = _indicators()
    xq_bf = xq_np.astype(bf).reshape(B, 2, 128, N)
    xs_bf = xs_np.astype(bf).reshape(B, 2, 128, N)

    in_maps = []
    for i in range(NCORES):
        sl = slice(i * S, (i + 1) * S)
        a4 = arn_c[:, sl].reshape(16, 2, 128, S)
        arn_sh = np.ascontiguousarray(
            a4.transpose(0, 2, 1, 3).reshape(16, 128, 2 * S))
        in_maps.append({
            "xq": np.ascontiguousarray(xq_bf[:, :, :, sl]),
            "xs": np.ascontiguousarray(xs_bf[:, :, :, sl]),
            "arn": arn_sh,
            "wkv": wkv_h.astype(bf), "wq": wq_h.astype(bf),
            "bkv": bkv_h.astype(bf), "bq": bq_h,
            "i4_in": i4.astype(bf), "sel_in": sel.astype(bf),
            "ones_c": np.ones((128, 1), bf),
            "ones_r": np.ones((1, 128), bf),
        })
    return in_maps


def kernel(**inputs):
    if "nc" not in _CACHE:
        _CACHE["nc"] = _build()
    nc = _CACHE["nc"]
    in_maps = _prep_inputs(**inputs)
    res = run_bass_kernel_spmd(nc, in_maps, list(range(NCORES)))
    full = np.empty((B, D, N), np.float32)
    for i in range(NCORES):
        full[:, :, i * S:(i + 1) * S] = res.results[i]["out"]
    return full
